# revision 5
# baseline (speedup 1.0000x reference)
"""AttnBlock (GroupNorm + single-head 1x1-conv attention + residual) on 8 TRN2 cores.

Data-parallel over batch: core i processes x[i] (512, 64*64) entirely on-chip.

Math (per batch item, N = 64*64 = 4096 spatial positions, C = 512 channels):
  R = groupnorm(x)                          [C, N]
  scores = (Wq R)^T (Wk R) / sqrt(C) = R^T Ws R / sqrt(C),  Ws = Wq^T Wk (host)
  attn   = softmax(scores, axis=m)
  out    = x + Wp (V attn^T) + pb,  V = Wk R + kb
Host-side folds: proj into V (V' = (Wp Wk) R), Wp kb into the output bias.
All big matmuls run fp8e4m3 DoubleRow (256-deep contraction); fp32 PSUM
accumulation; GroupNorm stats fp32 (rsqrt via bit-trick + Newton on DVE so
ScalarE only runs Exp/Copy -> no activation-table swaps).

One-shot schedule (the graded path):
  1. GN head: DMA x (8.4MB) saturating the queues, bn_stats trailing per
     slice, per-group aggregate via tiny PE matmuls, normalize -> r8 fp8.
  2. U/V' phase: 128 DR matmuls through a 6-bank PSUM staging pool
     (PE back-to-back), PSUM->SBUF fp8 evacuations split DVE/ScalarE,
     emitted in the order the attention loop consumes them.
  3. Attention: per n-chunk (8 x 512 cols), 16 m-pair iterations: 4 score
     MMs into a rotating score-bank ring, 2 exp ACTs -> fp8 et, and
     DEPTH-lagged PV accumulation (4 MMs) into 4 pso banks.  The softmax
     denominator is accumulated on DVE (pair-sum + f32 accumulate, off the
     critical ring) and closed per n-chunk with one fp32 ones-matmul --
     saving the 128 DR denominator matmuls (~30us of PE) at ~21us/nch of
     spare DVE.  Output drain of n-chunk k-1 occupies the first pair slots
     of n-chunk k; the residual x tiles stream in mid-chunk.
HW-measured rates feeding this design: DR fp8 N=512 matmul ~225-270ns
(engine-doc/cost-model values are wrong), exp ACT 512-wide ~620ns, the
mixed MM+ACT attention pair ~2.0us with ACT fully hidden behind PE.
"""
import sys

sys.path.insert(0, "/opt/trn_rl_repo")

import numpy as np
import ml_dtypes

import concourse.bass as bass
import concourse.bacc as bacc
import concourse.mybir as mybir
import concourse.tile as tile
from concourse import bass_utils

F32 = mybir.dt.float32
I32 = mybir.dt.int32
BF16 = mybir.dt.bfloat16
FP8 = mybir.dt.float8e4
DR = mybir.MatmulPerfMode.DoubleRow
AF = mybir.ActivationFunctionType
OP = mybir.AluOpType

B = 8
C = 512
N = 4096          # 64*64 spatial
GROUPS = 32
GSIZE = 16        # channels per group
EPS = 1e-6
CCH = 4           # channel chunks of 128
NCH = 8           # n chunks of 512
MT = 32           # m tiles of 128
P = 128
NW = 512          # matmul free dim / n-chunk width
NPAIR = MT // 2
INV_SQRT_C = 1.0 / float(np.sqrt(C))

DVE_D = True      # softmax denominator on DVE instead of PE
SCORE_BANKS = 3   # score ring banks (2 or 3)
DEPTH = 3         # dpv lag in pairs

_BUILD_CACHE = {}


def _build(use_amt: bool, use_kb: bool, reps: int = 1):
    nc = bacc.Bacc("TRN2", target_bir_lowering=False)

    x_in = nc.dram_tensor("x_in", [C, N], F32, kind="ExternalInput")
    wst_d = nc.dram_tensor("wst", [C, C], FP8, kind="ExternalInput")
    wvt_d = nc.dram_tensor("wvt", [C, C], FP8, kind="ExternalInput")
    gamma_d = nc.dram_tensor("gamma_r", [P, CCH], F32, kind="ExternalInput")
    beta_d = nc.dram_tensor("beta_r", [P, CCH], F32, kind="ExternalInput")
    pb_d = nc.dram_tensor("pb_r", [P, CCH], F32, kind="ExternalInput")
    ones_d = nc.dram_tensor("ones_b", [P, 2 * P], FP8, kind="ExternalInput")
    g_d = nc.dram_tensor("gmat", [P, 8], F32, kind="ExternalInput")
    g2_d = nc.dram_tensor("g2mat", [8, P], F32, kind="ExternalInput")
    if use_amt:
        amtw_d = nc.dram_tensor("amtw", [P, CCH], FP8, kind="ExternalInput")
    out_d = nc.dram_tensor("out", [C, N], F32, kind="ExternalOutput")

    with tile.TileContext(nc) as tc:
        # ---- persistent pools ----
        const = tc.alloc_tile_pool(name="const", bufs=1)
        xs_pool = tc.alloc_tile_pool(name="xs_pool", bufs=2)
        r8_pool = tc.alloc_tile_pool(name="r8_pool", bufs=1)
        u8_pool = tc.alloc_tile_pool(name="u8_pool", bufs=1)
        vt_pool = tc.alloc_tile_pool(name="vt_pool", bufs=NPAIR)
        et_pool = tc.alloc_tile_pool(name="et_pool", bufs=12)
        xr_pool = tc.alloc_tile_pool(name="xr_pool", bufs=8)
        tt_pool = tc.alloc_tile_pool(name="tt_pool", bufs=4)
        ob_pool = tc.alloc_tile_pool(name="ob_pool", bufs=4)
        rd_pool = tc.alloc_tile_pool(name="rd_pool", bufs=2)
        bn_pool = tc.alloc_tile_pool(name="bn_pool", bufs=2)
        st_pool = tc.alloc_tile_pool(name="st_pool", bufs=2)
        dd_pool = tc.alloc_tile_pool(name="dd_pool", bufs=2)

        wst_sb = const.tile([P, CCH, NW], FP8)
        wvt_sb = const.tile([P, CCH, NW], FP8)
        gamma_sb = const.tile([P, CCH], F32)
        beta_sb = const.tile([P, CCH], F32)
        pb_sb = const.tile([P, CCH], F32)
        ones_sb = const.tile([P, 2, P], FP8)
        onesf_sb = const.tile([P, P], BF16)
        g_sb = const.tile([P, 8], F32)
        g2_sb = const.tile([8, P], F32)
        for cp in range(CCH):
            nc.sync.dma_start(out=wst_sb[:, cp, :], in_=wst_d[cp * P:(cp + 1) * P, :])
            nc.sync.dma_start(out=wvt_sb[:, cp, :], in_=wvt_d[cp * P:(cp + 1) * P, :])
        nc.sync.dma_start(out=gamma_sb, in_=gamma_d[:, :])
        nc.sync.dma_start(out=beta_sb, in_=beta_d[:, :])
        nc.sync.dma_start(out=pb_sb, in_=pb_d[:, :])
        nc.sync.dma_start(out=ones_sb, in_=ones_d[:, :].rearrange('p (a b) -> p a b', a=2))
        nc.vector.memset(onesf_sb, 1.0)
        nc.sync.dma_start(out=g_sb, in_=g_d[:, :])
        nc.sync.dma_start(out=g2_sb, in_=g2_d[:, :])
        if use_amt:
            amtw_sb = const.tile([P, CCH, 1], FP8)
            nc.sync.dma_start(out=amtw_sb[:, :, 0], in_=amtw_d[:, :])
            amt_sb = const.tile([P, MT], F32)

        # ================= per-rep emission =================

        def emit_rep():
            # ---------- 1. GroupNorm head ----------
            r8_sb = r8_pool.tile([P, CCH, N], FP8, tag="r8", name="r8")
            with tc.tile_pool(name="psg", bufs=1, space="PSUM") as psg_pool:
                x1s, s_sbs = [], []
                for cp in range(CCH):
                    x1 = xs_pool.tile([P, N], F32, tag="x1", name="x1")
                    for s in range(8):
                        nc.sync.dma_start(
                            out=x1[:, s * NW:(s + 1) * NW],
                            in_=x_in[cp * P:(cp + 1) * P, s * NW:(s + 1) * NW])
                    x1s.append(x1)
                for cp in range(CCH):
                    bnst = bn_pool.tile([P, 8, 6], F32, tag="bnst")
                    for s in range(8):
                        nc.vector.bn_stats(out=bnst[:, s, :],
                                           in_=x1s[cp][:, s * NW:(s + 1) * NW])
                    mv = bn_pool.tile([P, 2], F32, tag="mv")
                    nc.vector.bn_aggr(out=mv, in_=bnst)
                    # per-partition [mean, E[x^2]]
                    s_sb = bn_pool.tile([P, 2], F32, tag=f"s_sb{cp}")
                    nc.vector.tensor_copy(out=s_sb[:, 0:1], in_=mv[:, 0:1])
                    nc.vector.scalar_tensor_tensor(
                        out=s_sb[:, 1:2], in0=mv[:, 0:1], scalar=mv[:, 0:1],
                        in1=mv[:, 1:2], op0=OP.mult, op1=OP.add)
                    s_sbs.append(s_sb)
                for cp in range(CCH):
                    # group-aggregate via PE, rsqrt chain on DVE, broadcast
                    # back via PE, then normalize x -> r8 fp8
                    psg = psg_pool.tile([8, 2], F32, tag="psg", name="psg")
                    nc.tensor.matmul(psg, lhsT=g_sb, rhs=s_sbs[cp],
                                     start=True, stop=True)
                    mu = st_pool.tile([8, 1], F32, tag="mu")
                    nc.vector.tensor_scalar_mul(out=mu, in0=psg[:, 0:1],
                                                scalar1=1.0 / GSIZE)
                    ex2 = st_pool.tile([8, 1], F32, tag="ex2")
                    nc.vector.tensor_scalar_mul(out=ex2, in0=psg[:, 1:2],
                                                scalar1=1.0 / GSIZE)
                    musq = st_pool.tile([8, 1], F32, tag="musq")
                    nc.vector.tensor_mul(out=musq, in0=mu, in1=mu)
                    veps = st_pool.tile([8, 1], F32, tag="veps")
                    nc.vector.scalar_tensor_tensor(
                        out=veps, in0=ex2, scalar=EPS, in1=musq,
                        op0=OP.add, op1=OP.subtract)
                    # rsqrt seed: y0 = bits(0x5F3759DF - (v >> 1))
                    h_i = st_pool.tile([8, 1], I32, tag="h_i")
                    nc.vector.tensor_scalar(
                        out=h_i, in0=veps[:, :].bitcast(I32), scalar1=1,
                        scalar2=None, op0=OP.arith_shift_right)
                    y0_i = st_pool.tile([8, 1], I32, tag="y0_i")
                    nc.vector.tensor_scalar(
                        out=y0_i, in0=h_i, scalar1=-1, scalar2=0x5F3759DF,
                        op0=OP.mult, op1=OP.add)
                    y = y0_i[:, :].bitcast(F32)
                    for it in range(2):
                        t1 = st_pool.tile([8, 1], F32, tag=f"t1_{it}")
                        nc.vector.tensor_mul(out=t1, in0=y, in1=y)
                        t2 = st_pool.tile([8, 1], F32, tag=f"t2_{it}")
                        nc.vector.tensor_mul(out=t2, in0=t1, in1=veps)
                        t3 = st_pool.tile([8, 1], F32, tag=f"t3_{it}")
                        nc.vector.tensor_scalar(
                            out=t3, in0=t2, scalar1=-0.5, scalar2=1.5,
                            op0=OP.mult, op1=OP.add)
                        yn = st_pool.tile([8, 1], F32, tag=f"yn_{it}")
                        nc.vector.tensor_mul(out=yn, in0=t3, in1=y)
                        y = yn
                    w_sb = st_pool.tile([8, 2], F32, tag="w_sb")
                    nc.vector.tensor_copy(out=w_sb[:, 0:1], in_=y)
                    nc.vector.tensor_copy(out=w_sb[:, 1:2], in_=mu)
                    psp2 = psg_pool.tile([P, 2], F32, tag="psg", name="psp2")
                    nc.tensor.matmul(psp2, lhsT=g2_sb, rhs=w_sb,
                                     start=True, stop=True)
                    a_c = st_pool.tile([P, 1], F32, tag="a_c")
                    nc.vector.tensor_mul(out=a_c, in0=gamma_sb[:, cp:cp + 1],
                                         in1=psp2[:, 0:1])
                    tb = st_pool.tile([P, 1], F32, tag="tb")
                    nc.vector.tensor_mul(out=tb, in0=psp2[:, 1:2], in1=a_c)
                    b_c = st_pool.tile([P, 1], F32, tag="b_c")
                    nc.vector.tensor_sub(out=b_c, in0=beta_sb[:, cp:cp + 1],
                                         in1=tb)
                    nc.vector.tensor_scalar(out=r8_sb[:, cp, :], in0=x1s[cp],
                                            scalar1=a_c, scalar2=b_c,
                                            op0=OP.mult, op1=OP.add)

            # ---------- 2. U / V' phase (multi-bank staging) ----------
            u8_sb = u8_pool.tile([P, CCH, N], FP8, tag="u8", name="u8")
            vt_sb = []
            with tc.tile_pool(name="psv", bufs=6, space="PSUM") as psv_pool:
                copy_i = [0]

                def evac(dst, src):
                    # alternate evacuation engine so neither paces PE
                    if copy_i[0] % 2 == 0:
                        nc.vector.tensor_copy(out=dst, in_=src)
                    else:
                        nc.scalar.activation(out=dst, in_=src, func=AF.Copy)
                    copy_i[0] += 1

                def emit_u(cq, mc):
                    psv = psv_pool.tile([P, NW], F32, tag="psv", name="psv")
                    for ks in (0, 2):
                        nc.tensor.matmul(
                            psv,
                            lhsT=wst_sb[:, ks:ks + 2, cq * P:(cq + 1) * P],
                            rhs=r8_sb[:, ks:ks + 2, mc * NW:(mc + 1) * NW],
                            start=(ks == 0), stop=(ks == 2), perf_mode=DR)
                    evac(u8_sb[:, cq, mc * NW:(mc + 1) * NW], psv)

                def emit_v(mt):
                    if mt % 2 == 0:
                        vt_sb.append(vt_pool.tile([P, 2, NW], FP8, tag="vt",
                                                  name="vt"))
                    psv = psv_pool.tile([P, NW], F32, tag="psv", name="psv")
                    for ks in (0, 2):
                        nc.tensor.matmul(
                            psv,
                            lhsT=r8_sb[:, ks:ks + 2, mt * P:(mt + 1) * P],
                            rhs=wvt_sb[:, ks:ks + 2, :],
                            start=(ks == 0), stop=(ks == 2), perf_mode=DR)
                    evac(vt_sb[mt // 2][:, mt % 2, :], psv)
                    if use_amt:
                        psa = psv_pool.tile([P, 1], F32, tag="psa", name="psa")
                        for ks in (0, 2):
                            nc.tensor.matmul(
                                psa,
                                lhsT=r8_sb[:, ks:ks + 2, mt * P:(mt + 1) * P],
                                rhs=amtw_sb[:, ks:ks + 2, :],
                                start=(ks == 0), stop=(ks == 2), perf_mode=DR)
                        nc.vector.tensor_copy(out=amt_sb[:, mt:mt + 1], in_=psa)

                # consumption order: scores consume u8 window mc at pair 2*mc;
                # dpv consumes vt[pt] at pair pt+DEPTH.
                for mc in range(NCH):
                    for cq in range(CCH):
                        emit_u(cq, mc)
                    for mt in (4 * mc, 4 * mc + 1, 4 * mc + 2, 4 * mc + 3):
                        emit_v(mt)

            # ---------- 3. attention ----------
            with tc.tile_pool(name="pss", bufs=SCORE_BANKS, space="PSUM") as pss_pool, \
                 tc.tile_pool(name="pso", bufs=1, space="PSUM") as pso_pool, \
                 tc.tile_pool(name="psd", bufs=1, space="PSUM") as psd_pool:

                def emit_dpv(et_t, pt, psd_t, pso_tiles, first, last):
                    if not DVE_D:
                        nc.tensor.matmul(psd_t, lhsT=ones_sb, rhs=et_t,
                                         start=first, stop=last, perf_mode=DR)
                    for cs in range(CCH):
                        nc.tensor.matmul(
                            pso_tiles[cs],
                            lhsT=vt_sb[pt][:, :, cs * P:(cs + 1) * P],
                            rhs=et_t, start=first, stop=last, perf_mode=DR)

                xr_tiles = {}

                def emit_out(state, cs):
                    pso_tiles, rd_t, pnch = state
                    t_t = tt_pool.tile([P, NW], F32, tag="t_t")
                    nc.vector.tensor_mul(out=t_t, in0=pso_tiles[cs], in1=rd_t)
                    ob = ob_pool.tile([P, NW], F32, tag="ob")
                    nc.vector.scalar_tensor_tensor(
                        out=ob, in0=t_t, scalar=pb_sb[:, cs:cs + 1],
                        in1=xr_tiles.pop((pnch, cs)),
                        op0=OP.add, op1=OP.add)
                    nc.sync.dma_start(
                        out=out_d[cs * P:(cs + 1) * P, pnch * NW:(pnch + 1) * NW],
                        in_=ob)

                state = None
                out_slots = {1: 0, 2: 1, 3: 2, 4: 3}
                xr_slots = {16: 0, 18: 1, 20: 2, 22: 3}
                for nch in range(NCH):
                    pso_tiles = [pso_pool.tile([P, NW], F32, tag=f"pso{cs}",
                                               name=f"pso{cs}")
                                 for cs in range(CCH)]
                    psd_t = None
                    if not DVE_D:
                        psd_t = psd_pool.tile([P, NW], F32, tag="psd")
                    d_acc = None
                    pend = []
                    cur_et = None
                    pss_pair = [None, None]
                    for mt in range(MT):
                        half = mt % 2
                        if half == 0:
                            # pair of score banks; ks-passes interleaved so
                            # consecutive matmuls never hit the same PSUM
                            # bank back-to-back (HW RMW hazard)
                            pss_pair[0] = pss_pool.tile([P, NW], F32,
                                                        tag="pss", name="pssA")
                            pss_pair[1] = pss_pool.tile([P, NW], F32,
                                                        tag="pss", name="pssB")
                            for ks in (0, 2):
                                for h in (0, 1):
                                    nc.tensor.matmul(
                                        pss_pair[h],
                                        lhsT=u8_sb[:, ks:ks + 2,
                                                   (mt + h) * P:(mt + h + 1) * P],
                                        rhs=r8_sb[:, ks:ks + 2,
                                                  nch * NW:(nch + 1) * NW],
                                        start=(ks == 0), stop=(ks == 2),
                                        perf_mode=DR)
                            cur_et = et_pool.tile([P, 2, NW], FP8, tag="et",
                                                  name="et")
                        pss = pss_pair[half]
                        if use_amt:
                            nc.scalar.activation(out=cur_et[:, half, :], in_=pss,
                                                 func=AF.Exp, scale=INV_SQRT_C,
                                                 bias=amt_sb[:, mt:mt + 1])
                        else:
                            nc.scalar.activation(out=cur_et[:, half, :], in_=pss,
                                                 func=AF.Exp, scale=INV_SQRT_C)
                        if DVE_D and half == 1:
                            # denominator partials on DVE, off the ring.
                            # bf16 accumulation keeps the adds in the DVE 2x
                            # perf mode; d's rounding error (~0.1-0.3% rel)
                            # scales whole output columns and sits well
                            # inside the error budget.
                            if d_acc is None:
                                d_acc = dd_pool.tile([P, NW], BF16, tag="d_acc0")
                                nc.vector.scalar_tensor_tensor(
                                    out=d_acc, in0=cur_et[:, 0, :], scalar=1.0,
                                    in1=cur_et[:, 1, :], op0=OP.mult, op1=OP.add)
                            else:
                                psum_t = dd_pool.tile([P, NW], BF16, tag="ps_d")
                                nc.vector.scalar_tensor_tensor(
                                    out=psum_t, in0=cur_et[:, 0, :], scalar=1.0,
                                    in1=cur_et[:, 1, :], op0=OP.mult, op1=OP.add)
                                d_new = dd_pool.tile([P, NW], BF16,
                                                     tag=f"d_acc{(mt // 2) % 2}")
                                nc.vector.tensor_add(out=d_new, in0=d_acc,
                                                     in1=psum_t)
                                d_acc = d_new
                        if state is not None and mt in out_slots:
                            emit_out(state, out_slots[mt])
                        if mt in xr_slots:
                            cs = xr_slots[mt]
                            xr = xr_pool.tile([P, NW], F32, tag="xr")
                            nc.sync.dma_start(
                                out=xr,
                                in_=x_in[cs * P:(cs + 1) * P,
                                         nch * NW:(nch + 1) * NW])
                            xr_tiles[(nch, cs)] = xr
                        if half == 1:
                            pend.append((cur_et, mt // 2))
                            if len(pend) > DEPTH:
                                p_et, pt = pend.pop(0)
                                emit_dpv(p_et, pt, psd_t, pso_tiles,
                                         first=(pt == 0), last=False)
                    for p_et, pt in pend:
                        emit_dpv(p_et, pt, psd_t, pso_tiles,
                                 first=(pt == 0), last=(pt == NPAIR - 1))
                    rd_t = rd_pool.tile([P, NW], F32, tag="rd")
                    if DVE_D:
                        psd2 = psd_pool.tile([P, NW], F32, tag="psd")
                        nc.tensor.matmul(psd2, lhsT=onesf_sb, rhs=d_acc,
                                         start=True, stop=True)
                        nc.vector.reciprocal(out=rd_t, in_=psd2)
                    else:
                        nc.vector.reciprocal(out=rd_t, in_=psd_t)
                    state = (pso_tiles, rd_t, nch)
                for cs in range(CCH):
                    emit_out(state, cs)

        for _rep in range(reps):
            emit_rep()

        for pool in (dd_pool, st_pool, bn_pool, rd_pool, ob_pool, tt_pool,
                     xr_pool, et_pool, vt_pool, u8_pool, r8_pool, xs_pool,
                     const):
            pool.release()

    nc.compile()
    return nc


def _prep_inputs(x, gn_gamma, gn_beta, q_w, q_b, k_w, k_b, proj_w, proj_b):
    use_amt = bool(np.any(q_b != 0))

    f8 = ml_dtypes.float8_e4m3
    f64 = np.float64
    ws_t = np.ascontiguousarray((k_w.T.astype(f64) @ q_w.astype(f64))
                                .astype(np.float32).astype(f8))
    # V' = (Wp Wk) R ; wvt = (Wp Wk)^T = Wk^T Wp^T
    wv_t = np.ascontiguousarray((k_w.T.astype(f64) @ proj_w.T.astype(f64))
                                .astype(np.float32).astype(f8))
    gamma_r = np.ascontiguousarray(gn_gamma.reshape(CCH, P).T.astype(np.float32))
    beta_r = np.ascontiguousarray(gn_beta.reshape(CCH, P).T.astype(np.float32))
    # attn rows sum to 1, so Wp kb is a constant channel bias: fold into pb
    pb_eff = (proj_b.astype(f64) + proj_w.astype(f64) @ k_b.astype(f64)).astype(
        np.float32)
    pb_r = np.ascontiguousarray(pb_eff.reshape(CCH, P).T)
    ones_b = np.ones((P, 2 * P), dtype=f8)
    gmat = np.zeros((P, 8), dtype=np.float32)
    gmat[np.arange(P), np.arange(P) // GSIZE] = 1.0
    g2mat = np.ascontiguousarray(gmat.T)

    common = {
        "wst": ws_t, "wvt": wv_t,
        "gamma_r": gamma_r, "beta_r": beta_r, "pb_r": pb_r,
        "ones_b": ones_b, "gmat": gmat, "g2mat": g2mat,
    }
    if use_amt:
        # amt[m] = qb . (Wk r_m) = (Wk^T qb) . r_m, pre-scaled by 1/sqrt(C);
        # the qb.kb term is constant over m AND n -> cancels in softmax.
        w_vec = (k_w.T.astype(f64) @ q_b.astype(f64)).astype(np.float32)
        w_vec = w_vec * INV_SQRT_C
        common["amtw"] = np.ascontiguousarray(
            w_vec.reshape(CCH, P).T.astype(f8))

    in_maps = []
    for i in range(B):
        m = dict(common)
        m["x_in"] = np.ascontiguousarray(x[i].reshape(C, N).astype(np.float32))
        in_maps.append(m)
    return in_maps, use_amt, False


def kernel(x, gn_gamma, gn_beta, q_w, q_b, k_w, k_b, proj_w, proj_b, _trace=False):
    x = np.asarray(x)
    in_maps, use_amt, use_kb = _prep_inputs(
        x, np.asarray(gn_gamma), np.asarray(gn_beta), np.asarray(q_w),
        np.asarray(q_b), np.asarray(k_w), np.asarray(k_b),
        np.asarray(proj_w), np.asarray(proj_b))

    key = (use_amt, use_kb)
    if key not in _BUILD_CACHE:
        _BUILD_CACHE[key] = _build(use_amt, use_kb)
    nc = _BUILD_CACHE[key]

    res = bass_utils.run_bass_kernel_spmd(
        nc, in_maps, core_ids=list(range(B)), trace=_trace)
    out = np.stack([r["out"].reshape(C, 64, 64) for r in res.results])
    kernel.last_result = res
    return out.astype(x.dtype)


def make_runner(inputs, chain=1):
    """Build the jitted 8-core executable once; return a callable that runs it
    once and returns wall ns, plus a decoder for the outputs."""
    import time
    import jax
    from jax.experimental.shard_map import shard_map
    from jax.sharding import Mesh, PartitionSpec
    from concourse import bass2jax
    import concourse.mybir as mb

    in_maps, use_amt, use_kb = _prep_inputs(
        np.asarray(inputs["x"]), np.asarray(inputs["gn_gamma"]),
        np.asarray(inputs["gn_beta"]), np.asarray(inputs["q_w"]),
        np.asarray(inputs["q_b"]), np.asarray(inputs["k_w"]),
        np.asarray(inputs["k_b"]), np.asarray(inputs["proj_w"]),
        np.asarray(inputs["proj_b"]))
    key = (use_amt, use_kb, chain)
    if key not in _BUILD_CACHE:
        _BUILD_CACHE[key] = _build(use_amt, use_kb, reps=chain)
    nc = _BUILD_CACHE[key]

    bass2jax.install_neuronx_cc_hook()
    partition_name = nc.partition_id_tensor.name if nc.partition_id_tensor else None
    in_names, out_names, out_avals, zero_outs = [], [], [], []
    for alloc in nc.m.functions[0].allocations:
        if not isinstance(alloc, mb.MemoryLocationSet):
            continue
        name = alloc.memorylocations[0].name
        if alloc.kind == "ExternalInput":
            if name != partition_name:
                in_names.append(name)
        elif alloc.kind == "ExternalOutput":
            out_names.append(name)
            shape = tuple(alloc.tensor_shape)
            dtype = mb.dt.np(alloc.dtype)
            out_avals.append(jax.core.ShapedArray(shape, dtype))
            zero_outs.append(np.zeros(shape, dtype))
    n_params = len(in_names)
    n_outs = len(out_avals)
    all_names = in_names + out_names
    if partition_name is not None:
        all_names = all_names + [partition_name]

    def _body(*args):
        operands = list(args)
        if partition_name is not None:
            operands.append(bass2jax.partition_id_tensor())
        outs = bass2jax._bass_exec_p.bind(
            *operands,
            out_avals=tuple(out_avals),
            in_names=tuple(all_names),
            out_names=tuple(out_names),
            lowering_input_output_aliases=(),
            sim_require_finite=True,
            sim_require_nnan=True,
            nc=nc,
        )
        return tuple(outs)

    donate = tuple(range(n_params, n_params + n_outs))
    devices = jax.devices()[:B]
    mesh = Mesh(np.asarray(devices), ("core",))
    sharded = jax.jit(
        shard_map(_body, mesh=mesh,
                  in_specs=(PartitionSpec("core"),) * (n_params + n_outs),
                  out_specs=(PartitionSpec("core"),) * n_outs,
                  check_rep=False),
        donate_argnums=donate, keep_unused=True)

    concat_in = [
        np.concatenate([np.asarray(in_maps[c][nm]) for c in range(B)], axis=0)
        for nm in in_names
    ]
    concat_zeros = [
        np.zeros((B * z.shape[0], *z.shape[1:]), z.dtype) for z in zero_outs
    ]
    sharding = jax.sharding.NamedSharding(mesh, PartitionSpec("core"))
    dev_in = [jax.device_put(a, sharding) for a in concat_in]

    state = {}

    def run_once():
        dev_zeros = [jax.device_put(z, sharding) for z in concat_zeros]
        for z in dev_zeros:
            z.block_until_ready()
        t0 = time.perf_counter()
        out_arrs = sharded(*dev_in, *dev_zeros)
        for o in out_arrs:
            o.block_until_ready()
        dt = (time.perf_counter() - t0) * 1e9
        state["out_arrs"] = out_arrs
        return dt

    def decode():
        out_arrs = state["out_arrs"]
        return [
            {nm: np.asarray(out_arrs[i]).reshape(B, *out_avals[i].shape)[c]
             for i, nm in enumerate(out_names)}
            for c in range(B)
        ]

    return run_once, decode


def bench(inputs, iters=6, chain=1):
    run_once, decode = make_runner(inputs, chain=chain)
    times = [run_once() for _ in range(iters)]
    return min(times), times, decode()


# revision 8
# speedup vs baseline: 1.2500x; 1.2500x over previous
"""AttnBlock (GroupNorm + single-head 1x1-conv attention + residual) on 8 TRN2 cores.

Data-parallel over batch: core i processes x[i] (512, 64*64) entirely on-chip.

Math (per batch item, N = 64*64 = 4096 spatial positions, C = 512 channels):
  R = groupnorm(x)                          [C, N]
  scores = (Wq R)^T (Wk R) / sqrt(C) = R^T Ws R / sqrt(C),  Ws = Wq^T Wk (host)
  attn   = softmax(scores, axis=m)
  out    = x + Wp (V attn^T) + pb,  V = Wk R + kb
Host-side folds: proj into V (V' = (Wp Wk) R), Wp kb into the output bias.
All big matmuls run fp8e4m3 DoubleRow (256-deep contraction); fp32 PSUM
accumulation; GroupNorm stats fp32 (rsqrt via bit-trick + Newton on DVE so
ScalarE only runs Exp/Copy -> no activation-table swaps).

One-shot schedule (the graded path):
  1. GN head: DMA x (8.4MB) saturating the queues, bn_stats trailing per
     slice, per-group aggregate via tiny PE matmuls, normalize -> r8 fp8.
  2. U/V' phase: 128 DR matmuls through a 6-bank PSUM staging pool
     (PE back-to-back), PSUM->SBUF fp8 evacuations split DVE/ScalarE,
     emitted in the order the attention loop consumes them.
  3. Attention: per n-chunk (8 x 512 cols), 16 m-pair iterations: 4 score
     MMs into a rotating score-bank ring, 2 exp ACTs -> fp8 et, and
     DEPTH-lagged PV accumulation (4 MMs) into 4 pso banks.  The softmax
     denominator is accumulated on DVE (pair-sum + f32 accumulate, off the
     critical ring) and closed per n-chunk with one fp32 ones-matmul --
     saving the 128 DR denominator matmuls (~30us of PE) at ~21us/nch of
     spare DVE.  Output drain of n-chunk k-1 occupies the first pair slots
     of n-chunk k; the residual x tiles stream in mid-chunk.
HW-measured rates feeding this design: DR fp8 N=512 matmul ~225-270ns
(engine-doc/cost-model values are wrong), exp ACT 512-wide ~620ns, the
mixed MM+ACT attention pair ~2.0us with ACT fully hidden behind PE.
"""
import sys

sys.path.insert(0, "/opt/trn_rl_repo")

import numpy as np
import ml_dtypes

import concourse.bass as bass
import concourse.bacc as bacc
import concourse.mybir as mybir
import concourse.tile as tile
from concourse import bass_utils

F32 = mybir.dt.float32
I32 = mybir.dt.int32
BF16 = mybir.dt.bfloat16
FP8 = mybir.dt.float8e4
DR = mybir.MatmulPerfMode.DoubleRow
AF = mybir.ActivationFunctionType
OP = mybir.AluOpType

B = 8
C = 512
N = 4096          # 64*64 spatial
GROUPS = 32
GSIZE = 16        # channels per group
EPS = 1e-6
CCH = 4           # channel chunks of 128
NCH = 8           # n chunks of 512
MT = 32           # m tiles of 128
P = 128
NW = 512          # matmul free dim / n-chunk width
NPAIR = MT // 2
INV_SQRT_C = 1.0 / float(np.sqrt(C))

DVE_D = True      # softmax denominator on DVE instead of PE
SCORE_BANKS = 3   # score ring banks (2 or 3)
DEPTH = 3         # dpv lag in pairs

_BUILD_CACHE = {}


def _build(use_amt: bool, use_kb: bool, reps: int = 1):
    nc = bacc.Bacc("TRN2", target_bir_lowering=False)

    x_in = nc.dram_tensor("x_in", [C, N], F32, kind="ExternalInput")
    wst_d = nc.dram_tensor("wst", [C, C], FP8, kind="ExternalInput")
    wvt_d = nc.dram_tensor("wvt", [C, C], FP8, kind="ExternalInput")
    gamma_d = nc.dram_tensor("gamma_r", [P, CCH], F32, kind="ExternalInput")
    beta_d = nc.dram_tensor("beta_r", [P, CCH], F32, kind="ExternalInput")
    pb_d = nc.dram_tensor("pb_r", [P, CCH], F32, kind="ExternalInput")
    ones_d = nc.dram_tensor("ones_b", [P, 2 * P], FP8, kind="ExternalInput")
    g_d = nc.dram_tensor("gmat", [P, 8], F32, kind="ExternalInput")
    g2_d = nc.dram_tensor("g2mat", [8, P], F32, kind="ExternalInput")
    if use_amt:
        amtw_d = nc.dram_tensor("amtw", [P, CCH], FP8, kind="ExternalInput")
    out_d = nc.dram_tensor("out", [C, N], F32, kind="ExternalOutput")

    with tile.TileContext(nc) as tc:
        # ---- persistent pools ----
        const = tc.alloc_tile_pool(name="const", bufs=1)
        xs_pool = tc.alloc_tile_pool(name="xs_pool", bufs=2)
        r8_pool = tc.alloc_tile_pool(name="r8_pool", bufs=1)
        u8_pool = tc.alloc_tile_pool(name="u8_pool", bufs=1)
        vt_pool = tc.alloc_tile_pool(name="vt_pool", bufs=NPAIR)
        et_pool = tc.alloc_tile_pool(name="et_pool", bufs=12)
        xr_pool = tc.alloc_tile_pool(name="xr_pool", bufs=8)
        tt_pool = tc.alloc_tile_pool(name="tt_pool", bufs=4)
        ob_pool = tc.alloc_tile_pool(name="ob_pool", bufs=4)
        rd_pool = tc.alloc_tile_pool(name="rd_pool", bufs=2)
        bn_pool = tc.alloc_tile_pool(name="bn_pool", bufs=2)
        st_pool = tc.alloc_tile_pool(name="st_pool", bufs=2)
        dd_pool = tc.alloc_tile_pool(name="dd_pool", bufs=2)

        wst_sb = const.tile([P, CCH, NW], FP8)
        wvt_sb = const.tile([P, CCH, NW], FP8)
        gamma_sb = const.tile([P, CCH], F32)
        beta_sb = const.tile([P, CCH], F32)
        pb_sb = const.tile([P, CCH], F32)
        ones_sb = const.tile([P, 2, P], FP8)
        onesf_sb = const.tile([P, P], BF16)
        g_sb = const.tile([P, 8], F32)
        g2_sb = const.tile([8, P], F32)
        for cp in range(CCH):
            nc.sync.dma_start(out=wst_sb[:, cp, :], in_=wst_d[cp * P:(cp + 1) * P, :])
            nc.sync.dma_start(out=wvt_sb[:, cp, :], in_=wvt_d[cp * P:(cp + 1) * P, :])
        nc.sync.dma_start(out=gamma_sb, in_=gamma_d[:, :])
        nc.sync.dma_start(out=beta_sb, in_=beta_d[:, :])
        nc.sync.dma_start(out=pb_sb, in_=pb_d[:, :])
        nc.sync.dma_start(out=ones_sb, in_=ones_d[:, :].rearrange('p (a b) -> p a b', a=2))
        nc.vector.memset(onesf_sb, 1.0)
        nc.sync.dma_start(out=g_sb, in_=g_d[:, :])
        nc.sync.dma_start(out=g2_sb, in_=g2_d[:, :])
        if use_amt:
            amtw_sb = const.tile([P, CCH, 1], FP8)
            nc.sync.dma_start(out=amtw_sb[:, :, 0], in_=amtw_d[:, :])
            amt_sb = const.tile([P, MT], F32)

        # ================= per-rep emission =================

        def emit_rep():
            # ---------- 1. GroupNorm head ----------
            r8_sb = r8_pool.tile([P, CCH, N], FP8, tag="r8", name="r8")
            with tc.tile_pool(name="psg", bufs=1, space="PSUM") as psg_pool:
                x1s, s_sbs = [], []
                for cp in range(CCH):
                    x1 = xs_pool.tile([P, N], F32, tag="x1", name="x1")
                    for s in range(8):
                        nc.sync.dma_start(
                            out=x1[:, s * NW:(s + 1) * NW],
                            in_=x_in[cp * P:(cp + 1) * P, s * NW:(s + 1) * NW])
                    x1s.append(x1)
                for cp in range(CCH):
                    bnst = bn_pool.tile([P, 8, 6], F32, tag="bnst")
                    for s in range(8):
                        nc.vector.bn_stats(out=bnst[:, s, :],
                                           in_=x1s[cp][:, s * NW:(s + 1) * NW])
                    mv = bn_pool.tile([P, 2], F32, tag="mv")
                    nc.vector.bn_aggr(out=mv, in_=bnst)
                    # per-partition [mean, E[x^2]]
                    s_sb = bn_pool.tile([P, 2], F32, tag=f"s_sb{cp}")
                    nc.vector.tensor_copy(out=s_sb[:, 0:1], in_=mv[:, 0:1])
                    nc.vector.scalar_tensor_tensor(
                        out=s_sb[:, 1:2], in0=mv[:, 0:1], scalar=mv[:, 0:1],
                        in1=mv[:, 1:2], op0=OP.mult, op1=OP.add)
                    s_sbs.append(s_sb)
                for cp in range(CCH):
                    # group-aggregate via PE, rsqrt chain on DVE, broadcast
                    # back via PE, then normalize x -> r8 fp8
                    psg = psg_pool.tile([8, 2], F32, tag="psg", name="psg")
                    nc.tensor.matmul(psg, lhsT=g_sb, rhs=s_sbs[cp],
                                     start=True, stop=True)
                    mu = st_pool.tile([8, 1], F32, tag="mu")
                    nc.vector.tensor_scalar_mul(out=mu, in0=psg[:, 0:1],
                                                scalar1=1.0 / GSIZE)
                    ex2 = st_pool.tile([8, 1], F32, tag="ex2")
                    nc.vector.tensor_scalar_mul(out=ex2, in0=psg[:, 1:2],
                                                scalar1=1.0 / GSIZE)
                    musq = st_pool.tile([8, 1], F32, tag="musq")
                    nc.vector.tensor_mul(out=musq, in0=mu, in1=mu)
                    veps = st_pool.tile([8, 1], F32, tag="veps")
                    nc.vector.scalar_tensor_tensor(
                        out=veps, in0=ex2, scalar=EPS, in1=musq,
                        op0=OP.add, op1=OP.subtract)
                    # rsqrt seed: y0 = bits(0x5F3759DF - (v >> 1))
                    h_i = st_pool.tile([8, 1], I32, tag="h_i")
                    nc.vector.tensor_scalar(
                        out=h_i, in0=veps[:, :].bitcast(I32), scalar1=1,
                        scalar2=None, op0=OP.arith_shift_right)
                    y0_i = st_pool.tile([8, 1], I32, tag="y0_i")
                    nc.vector.tensor_scalar(
                        out=y0_i, in0=h_i, scalar1=-1, scalar2=0x5F3759DF,
                        op0=OP.mult, op1=OP.add)
                    y = y0_i[:, :].bitcast(F32)
                    for it in range(2):
                        t1 = st_pool.tile([8, 1], F32, tag=f"t1_{it}")
                        nc.vector.tensor_mul(out=t1, in0=y, in1=y)
                        t2 = st_pool.tile([8, 1], F32, tag=f"t2_{it}")
                        nc.vector.tensor_mul(out=t2, in0=t1, in1=veps)
                        t3 = st_pool.tile([8, 1], F32, tag=f"t3_{it}")
                        nc.vector.tensor_scalar(
                            out=t3, in0=t2, scalar1=-0.5, scalar2=1.5,
                            op0=OP.mult, op1=OP.add)
                        yn = st_pool.tile([8, 1], F32, tag=f"yn_{it}")
                        nc.vector.tensor_mul(out=yn, in0=t3, in1=y)
                        y = yn
                    w_sb = st_pool.tile([8, 2], F32, tag="w_sb")
                    nc.vector.tensor_copy(out=w_sb[:, 0:1], in_=y)
                    nc.vector.tensor_copy(out=w_sb[:, 1:2], in_=mu)
                    psp2 = psg_pool.tile([P, 2], F32, tag="psg", name="psp2")
                    nc.tensor.matmul(psp2, lhsT=g2_sb, rhs=w_sb,
                                     start=True, stop=True)
                    a_c = st_pool.tile([P, 1], F32, tag="a_c")
                    nc.vector.tensor_mul(out=a_c, in0=gamma_sb[:, cp:cp + 1],
                                         in1=psp2[:, 0:1])
                    tb = st_pool.tile([P, 1], F32, tag="tb")
                    nc.vector.tensor_mul(out=tb, in0=psp2[:, 1:2], in1=a_c)
                    b_c = st_pool.tile([P, 1], F32, tag="b_c")
                    nc.vector.tensor_sub(out=b_c, in0=beta_sb[:, cp:cp + 1],
                                         in1=tb)
                    nc.vector.tensor_scalar(out=r8_sb[:, cp, :], in0=x1s[cp],
                                            scalar1=a_c, scalar2=b_c,
                                            op0=OP.mult, op1=OP.add)

            # ---------- 2. U / V' phase (multi-bank staging) ----------
            u8_sb = u8_pool.tile([P, CCH, N], FP8, tag="u8", name="u8")
            vt_sb = []
            with tc.tile_pool(name="psv", bufs=6, space="PSUM") as psv_pool:
                copy_i = [0]

                def evac(dst, src):
                    # alternate evacuation engine so neither paces PE
                    if copy_i[0] % 2 == 0:
                        nc.vector.tensor_copy(out=dst, in_=src)
                    else:
                        nc.scalar.activation(out=dst, in_=src, func=AF.Copy)
                    copy_i[0] += 1

                def emit_u(cq, mc):
                    psv = psv_pool.tile([P, NW], F32, tag="psv", name="psv")
                    for ks in (0, 2):
                        nc.tensor.matmul(
                            psv,
                            lhsT=wst_sb[:, ks:ks + 2, cq * P:(cq + 1) * P],
                            rhs=r8_sb[:, ks:ks + 2, mc * NW:(mc + 1) * NW],
                            start=(ks == 0), stop=(ks == 2), perf_mode=DR)
                    evac(u8_sb[:, cq, mc * NW:(mc + 1) * NW], psv)

                def emit_v(mt):
                    if mt % 2 == 0:
                        vt_sb.append(vt_pool.tile([P, 2, NW], FP8, tag="vt",
                                                  name="vt"))
                    psv = psv_pool.tile([P, NW], F32, tag="psv", name="psv")
                    for ks in (0, 2):
                        nc.tensor.matmul(
                            psv,
                            lhsT=r8_sb[:, ks:ks + 2, mt * P:(mt + 1) * P],
                            rhs=wvt_sb[:, ks:ks + 2, :],
                            start=(ks == 0), stop=(ks == 2), perf_mode=DR)
                    evac(vt_sb[mt // 2][:, mt % 2, :], psv)
                    if use_amt:
                        psa = psv_pool.tile([P, 1], F32, tag="psa", name="psa")
                        for ks in (0, 2):
                            nc.tensor.matmul(
                                psa,
                                lhsT=r8_sb[:, ks:ks + 2, mt * P:(mt + 1) * P],
                                rhs=amtw_sb[:, ks:ks + 2, :],
                                start=(ks == 0), stop=(ks == 2), perf_mode=DR)
                        nc.vector.tensor_copy(out=amt_sb[:, mt:mt + 1], in_=psa)

                # consumption order: scores consume u8 window mc at pair 2*mc;
                # dpv consumes vt[pt] at pair pt+DEPTH.
                for mc in range(NCH):
                    for cq in range(CCH):
                        emit_u(cq, mc)
                    for mt in (4 * mc, 4 * mc + 1, 4 * mc + 2, 4 * mc + 3):
                        emit_v(mt)

            # ---------- 3. attention ----------
            with tc.tile_pool(name="pss", bufs=SCORE_BANKS, space="PSUM") as pss_pool, \
                 tc.tile_pool(name="pso", bufs=1, space="PSUM") as pso_pool, \
                 tc.tile_pool(name="psd", bufs=1, space="PSUM") as psd_pool:

                def emit_dpv(et_t, pt, psd_t, pso_tiles, first, last):
                    if not DVE_D:
                        nc.tensor.matmul(psd_t, lhsT=ones_sb, rhs=et_t,
                                         start=first, stop=last, perf_mode=DR)
                    for cs in range(CCH):
                        nc.tensor.matmul(
                            pso_tiles[cs],
                            lhsT=vt_sb[pt][:, :, cs * P:(cs + 1) * P],
                            rhs=et_t, start=first, stop=last, perf_mode=DR)

                xr_tiles = {}

                def emit_out(state, cs):
                    pso_tiles, rd_t, pnch = state
                    t_t = tt_pool.tile([P, NW], F32, tag="t_t")
                    nc.vector.tensor_mul(out=t_t, in0=pso_tiles[cs], in1=rd_t)
                    ob = ob_pool.tile([P, NW], F32, tag="ob")
                    nc.vector.scalar_tensor_tensor(
                        out=ob, in0=t_t, scalar=pb_sb[:, cs:cs + 1],
                        in1=xr_tiles.pop((pnch, cs)),
                        op0=OP.add, op1=OP.add)
                    nc.sync.dma_start(
                        out=out_d[cs * P:(cs + 1) * P, pnch * NW:(pnch + 1) * NW],
                        in_=ob)

                state = None
                dstate = None

                def close_d(pso_tiles_p, d_acc_p, psd_t_p, pnch):
                    # d-close: one small matmul broadcasts the partition-sum
                    # of the DVE denominator partials to all 128 rows
                    rd_t = rd_pool.tile([P, NW], F32, tag="rd")
                    if DVE_D:
                        psd2 = psd_pool.tile([P, NW], F32, tag="psd")
                        nc.tensor.matmul(psd2, lhsT=onesf_sb, rhs=d_acc_p,
                                         start=True, stop=True)
                        nc.vector.reciprocal(out=rd_t, in_=psd2)
                    else:
                        nc.vector.reciprocal(out=rd_t, in_=psd_t_p)
                    return (pso_tiles_p, rd_t, pnch)

                out_slots = {1: 0, 2: 1, 3: 2, 4: 3}
                xr_slots = {16: 0, 18: 1, 20: 2, 22: 3}
                for nch in range(NCH):
                    pso_tiles = [pso_pool.tile([P, NW], F32, tag=f"pso{cs}",
                                               name=f"pso{cs}")
                                 for cs in range(CCH)]
                    psd_t = None
                    if not DVE_D:
                        psd_t = psd_pool.tile([P, NW], F32, tag="psd")
                    d_acc = None
                    pend = []
                    cur_et = None
                    pss_pair = [None, None]
                    for mt in range(MT):
                        half = mt % 2
                        if half == 0:
                            # pair of score banks; ks-passes interleaved so
                            # consecutive matmuls never hit the same PSUM
                            # bank back-to-back (HW RMW hazard)
                            pss_pair[0] = pss_pool.tile([P, NW], F32,
                                                        tag="pss", name="pssA")
                            pss_pair[1] = pss_pool.tile([P, NW], F32,
                                                        tag="pss", name="pssB")
                            for ks in (0, 2):
                                for h in (0, 1):
                                    nc.tensor.matmul(
                                        pss_pair[h],
                                        lhsT=u8_sb[:, ks:ks + 2,
                                                   (mt + h) * P:(mt + h + 1) * P],
                                        rhs=r8_sb[:, ks:ks + 2,
                                                  nch * NW:(nch + 1) * NW],
                                        start=(ks == 0), stop=(ks == 2),
                                        perf_mode=DR)
                            cur_et = et_pool.tile([P, 2, NW], FP8, tag="et",
                                                  name="et")
                        pss = pss_pair[half]
                        if use_amt:
                            nc.scalar.activation(out=cur_et[:, half, :], in_=pss,
                                                 func=AF.Exp, scale=INV_SQRT_C,
                                                 bias=amt_sb[:, mt:mt + 1])
                        else:
                            nc.scalar.activation(out=cur_et[:, half, :], in_=pss,
                                                 func=AF.Exp, scale=INV_SQRT_C)
                        if DVE_D and half == 1:
                            # denominator partials on DVE, off the ring.
                            # bf16 accumulation keeps the adds in the DVE 2x
                            # perf mode; d's rounding error (~0.1-0.3% rel)
                            # scales whole output columns and sits well
                            # inside the error budget.
                            if d_acc is None:
                                d_acc = dd_pool.tile([P, NW], BF16, tag="d_acc0")
                                nc.vector.scalar_tensor_tensor(
                                    out=d_acc, in0=cur_et[:, 0, :], scalar=1.0,
                                    in1=cur_et[:, 1, :], op0=OP.mult, op1=OP.add)
                            else:
                                psum_t = dd_pool.tile([P, NW], BF16, tag="ps_d")
                                nc.vector.scalar_tensor_tensor(
                                    out=psum_t, in0=cur_et[:, 0, :], scalar=1.0,
                                    in1=cur_et[:, 1, :], op0=OP.mult, op1=OP.add)
                                d_new = dd_pool.tile([P, NW], BF16,
                                                     tag=f"d_acc{(mt // 2) % 2}")
                                nc.vector.tensor_add(out=d_new, in0=d_acc,
                                                     in1=psum_t)
                                d_acc = d_new
                        if mt == 0 and dstate is not None:
                            # deferred d-close of the previous n-chunk: by now
                            # the DVE d-chain tail has drained behind pair 0's
                            # score matmuls, so PE doesn't stall on it
                            state = close_d(*dstate)
                            dstate = None
                        if state is not None and mt in out_slots:
                            emit_out(state, out_slots[mt])
                        if mt in xr_slots:
                            cs = xr_slots[mt]
                            xr = xr_pool.tile([P, NW], F32, tag="xr")
                            nc.sync.dma_start(
                                out=xr,
                                in_=x_in[cs * P:(cs + 1) * P,
                                         nch * NW:(nch + 1) * NW])
                            xr_tiles[(nch, cs)] = xr
                        if half == 1:
                            pend.append((cur_et, mt // 2))
                            if len(pend) > DEPTH:
                                p_et, pt = pend.pop(0)
                                emit_dpv(p_et, pt, psd_t, pso_tiles,
                                         first=(pt == 0), last=False)
                    for p_et, pt in pend:
                        emit_dpv(p_et, pt, psd_t, pso_tiles,
                                 first=(pt == 0), last=(pt == NPAIR - 1))
                    dstate = (pso_tiles, d_acc, psd_t, nch)
                state = close_d(*dstate)
                for cs in range(CCH):
                    emit_out(state, cs)

        for _rep in range(reps):
            emit_rep()

        for pool in (dd_pool, st_pool, bn_pool, rd_pool, ob_pool, tt_pool,
                     xr_pool, et_pool, vt_pool, u8_pool, r8_pool, xs_pool,
                     const):
            pool.release()

    nc.compile()
    return nc


def _prep_inputs(x, gn_gamma, gn_beta, q_w, q_b, k_w, k_b, proj_w, proj_b):
    use_amt = bool(np.any(q_b != 0))

    f8 = ml_dtypes.float8_e4m3
    f64 = np.float64
    ws_t = np.ascontiguousarray((k_w.T.astype(f64) @ q_w.astype(f64))
                                .astype(np.float32).astype(f8))
    # V' = (Wp Wk) R ; wvt = (Wp Wk)^T = Wk^T Wp^T
    wv_t = np.ascontiguousarray((k_w.T.astype(f64) @ proj_w.T.astype(f64))
                                .astype(np.float32).astype(f8))
    gamma_r = np.ascontiguousarray(gn_gamma.reshape(CCH, P).T.astype(np.float32))
    beta_r = np.ascontiguousarray(gn_beta.reshape(CCH, P).T.astype(np.float32))
    # attn rows sum to 1, so Wp kb is a constant channel bias: fold into pb
    pb_eff = (proj_b.astype(f64) + proj_w.astype(f64) @ k_b.astype(f64)).astype(
        np.float32)
    pb_r = np.ascontiguousarray(pb_eff.reshape(CCH, P).T)
    ones_b = np.ones((P, 2 * P), dtype=f8)
    gmat = np.zeros((P, 8), dtype=np.float32)
    gmat[np.arange(P), np.arange(P) // GSIZE] = 1.0
    g2mat = np.ascontiguousarray(gmat.T)

    common = {
        "wst": ws_t, "wvt": wv_t,
        "gamma_r": gamma_r, "beta_r": beta_r, "pb_r": pb_r,
        "ones_b": ones_b, "gmat": gmat, "g2mat": g2mat,
    }
    if use_amt:
        # amt[m] = qb . (Wk r_m) = (Wk^T qb) . r_m, pre-scaled by 1/sqrt(C);
        # the qb.kb term is constant over m AND n -> cancels in softmax.
        w_vec = (k_w.T.astype(f64) @ q_b.astype(f64)).astype(np.float32)
        w_vec = w_vec * INV_SQRT_C
        common["amtw"] = np.ascontiguousarray(
            w_vec.reshape(CCH, P).T.astype(f8))

    in_maps = []
    for i in range(B):
        m = dict(common)
        m["x_in"] = np.ascontiguousarray(x[i].reshape(C, N).astype(np.float32))
        in_maps.append(m)
    return in_maps, use_amt, False


def kernel(x, gn_gamma, gn_beta, q_w, q_b, k_w, k_b, proj_w, proj_b, _trace=False):
    x = np.asarray(x)
    in_maps, use_amt, use_kb = _prep_inputs(
        x, np.asarray(gn_gamma), np.asarray(gn_beta), np.asarray(q_w),
        np.asarray(q_b), np.asarray(k_w), np.asarray(k_b),
        np.asarray(proj_w), np.asarray(proj_b))

    key = (use_amt, use_kb)
    if key not in _BUILD_CACHE:
        _BUILD_CACHE[key] = _build(use_amt, use_kb)
    nc = _BUILD_CACHE[key]

    res = bass_utils.run_bass_kernel_spmd(
        nc, in_maps, core_ids=list(range(B)), trace=_trace)
    out = np.stack([r["out"].reshape(C, 64, 64) for r in res.results])
    kernel.last_result = res
    return out.astype(x.dtype)


def make_runner(inputs, chain=1):
    """Build the jitted 8-core executable once; return a callable that runs it
    once and returns wall ns, plus a decoder for the outputs."""
    import time
    import jax
    from jax.experimental.shard_map import shard_map
    from jax.sharding import Mesh, PartitionSpec
    from concourse import bass2jax
    import concourse.mybir as mb

    in_maps, use_amt, use_kb = _prep_inputs(
        np.asarray(inputs["x"]), np.asarray(inputs["gn_gamma"]),
        np.asarray(inputs["gn_beta"]), np.asarray(inputs["q_w"]),
        np.asarray(inputs["q_b"]), np.asarray(inputs["k_w"]),
        np.asarray(inputs["k_b"]), np.asarray(inputs["proj_w"]),
        np.asarray(inputs["proj_b"]))
    key = (use_amt, use_kb, chain)
    if key not in _BUILD_CACHE:
        _BUILD_CACHE[key] = _build(use_amt, use_kb, reps=chain)
    nc = _BUILD_CACHE[key]

    bass2jax.install_neuronx_cc_hook()
    partition_name = nc.partition_id_tensor.name if nc.partition_id_tensor else None
    in_names, out_names, out_avals, zero_outs = [], [], [], []
    for alloc in nc.m.functions[0].allocations:
        if not isinstance(alloc, mb.MemoryLocationSet):
            continue
        name = alloc.memorylocations[0].name
        if alloc.kind == "ExternalInput":
            if name != partition_name:
                in_names.append(name)
        elif alloc.kind == "ExternalOutput":
            out_names.append(name)
            shape = tuple(alloc.tensor_shape)
            dtype = mb.dt.np(alloc.dtype)
            out_avals.append(jax.core.ShapedArray(shape, dtype))
            zero_outs.append(np.zeros(shape, dtype))
    n_params = len(in_names)
    n_outs = len(out_avals)
    all_names = in_names + out_names
    if partition_name is not None:
        all_names = all_names + [partition_name]

    def _body(*args):
        operands = list(args)
        if partition_name is not None:
            operands.append(bass2jax.partition_id_tensor())
        outs = bass2jax._bass_exec_p.bind(
            *operands,
            out_avals=tuple(out_avals),
            in_names=tuple(all_names),
            out_names=tuple(out_names),
            lowering_input_output_aliases=(),
            sim_require_finite=True,
            sim_require_nnan=True,
            nc=nc,
        )
        return tuple(outs)

    donate = tuple(range(n_params, n_params + n_outs))
    devices = jax.devices()[:B]
    mesh = Mesh(np.asarray(devices), ("core",))
    sharded = jax.jit(
        shard_map(_body, mesh=mesh,
                  in_specs=(PartitionSpec("core"),) * (n_params + n_outs),
                  out_specs=(PartitionSpec("core"),) * n_outs,
                  check_rep=False),
        donate_argnums=donate, keep_unused=True)

    concat_in = [
        np.concatenate([np.asarray(in_maps[c][nm]) for c in range(B)], axis=0)
        for nm in in_names
    ]
    concat_zeros = [
        np.zeros((B * z.shape[0], *z.shape[1:]), z.dtype) for z in zero_outs
    ]
    sharding = jax.sharding.NamedSharding(mesh, PartitionSpec("core"))
    dev_in = [jax.device_put(a, sharding) for a in concat_in]

    state = {}

    def run_once():
        dev_zeros = [jax.device_put(z, sharding) for z in concat_zeros]
        for z in dev_zeros:
            z.block_until_ready()
        t0 = time.perf_counter()
        out_arrs = sharded(*dev_in, *dev_zeros)
        for o in out_arrs:
            o.block_until_ready()
        dt = (time.perf_counter() - t0) * 1e9
        state["out_arrs"] = out_arrs
        return dt

    def decode():
        out_arrs = state["out_arrs"]
        return [
            {nm: np.asarray(out_arrs[i]).reshape(B, *out_avals[i].shape)[c]
             for i, nm in enumerate(out_names)}
            for c in range(B)
        ]

    return run_once, decode


def bench(inputs, iters=6, chain=1):
    run_once, decode = make_runner(inputs, chain=chain)
    times = [run_once() for _ in range(iters)]
    return min(times), times, decode()


# revision 10
# speedup vs baseline: 1.4686x; 1.1749x over previous
"""AttnBlock (GroupNorm + single-head 1x1-conv attention + residual) on 8 TRN2 cores.

Data-parallel over batch: core i processes x[i] (512, 64*64) entirely on-chip.

Math (per batch item, N = 64*64 = 4096 spatial positions, C = 512 channels):
  R = groupnorm(x)                          [C, N]
  scores = (Wq R)^T (Wk R) / sqrt(C) = R^T Ws R / sqrt(C),  Ws = Wq^T Wk (host)
  attn   = softmax(scores, axis=m)
  out    = x + Wp (V attn^T) + pb,  V = Wk R + kb
Host-side folds: proj into V (V' = (Wp Wk) R), Wp kb into the output bias.
All big matmuls run fp8e4m3 DoubleRow (256-deep contraction); fp32 PSUM
accumulation; GroupNorm stats fp32 (rsqrt via bit-trick + Newton on DVE so
ScalarE only runs Exp/Copy -> no activation-table swaps).

One-shot schedule (the graded path):
  1. GN head: DMA x (8.4MB) saturating the queues, bn_stats trailing per
     slice, per-group aggregate via tiny PE matmuls, normalize -> r8 fp8.
  2. U/V' phase: 128 DR matmuls through a 6-bank PSUM staging pool
     (PE back-to-back), PSUM->SBUF fp8 evacuations split DVE/ScalarE,
     emitted in the order the attention loop consumes them.
  3. Attention: per n-chunk (8 x 512 cols), 16 m-pair iterations: 4 score
     MMs into a rotating score-bank ring, 2 exp ACTs -> fp8 et, and
     DEPTH-lagged PV accumulation (4 MMs) into 4 pso banks.  The softmax
     denominator is accumulated on DVE (pair-sum + f32 accumulate, off the
     critical ring) and closed per n-chunk with one fp32 ones-matmul --
     saving the 128 DR denominator matmuls (~30us of PE) at ~21us/nch of
     spare DVE.  Output drain of n-chunk k-1 occupies the first pair slots
     of n-chunk k; the residual x tiles stream in mid-chunk.
HW-measured rates feeding this design: DR fp8 N=512 matmul ~225-270ns
(engine-doc/cost-model values are wrong), exp ACT 512-wide ~620ns, the
mixed MM+ACT attention pair ~2.0us with ACT fully hidden behind PE.
"""
import sys

sys.path.insert(0, "/opt/trn_rl_repo")

import numpy as np
import ml_dtypes

import concourse.bass as bass
import concourse.bacc as bacc
import concourse.mybir as mybir
import concourse.tile as tile
from concourse import bass_utils

F32 = mybir.dt.float32
I32 = mybir.dt.int32
BF16 = mybir.dt.bfloat16
FP8 = mybir.dt.float8e4
DR = mybir.MatmulPerfMode.DoubleRow
AF = mybir.ActivationFunctionType
OP = mybir.AluOpType

B = 8
C = 512
N = 4096          # 64*64 spatial
GROUPS = 32
GSIZE = 16        # channels per group
EPS = 1e-6
CCH = 4           # channel chunks of 128
NCH = 8           # n chunks of 512
MT = 32           # m tiles of 128
P = 128
NW = 512          # matmul free dim / n-chunk width
NPAIR = MT // 2
INV_SQRT_C = 1.0 / float(np.sqrt(C))

DVE_D = True      # softmax denominator on DVE instead of PE
SCORE_BANKS = 3   # score ring banks (2 or 3)
DEPTH = 3         # dpv lag in pairs

_BUILD_CACHE = {}


def _build(use_amt: bool, use_kb: bool, reps: int = 1):
    nc = bacc.Bacc("TRN2", target_bir_lowering=False)

    x_in = nc.dram_tensor("x_in", [C, N], F32, kind="ExternalInput")
    wst_d = nc.dram_tensor("wst", [C, C], FP8, kind="ExternalInput")
    wvt_d = nc.dram_tensor("wvt", [C, C], FP8, kind="ExternalInput")
    gamma_d = nc.dram_tensor("gamma_r", [P, CCH], F32, kind="ExternalInput")
    beta_d = nc.dram_tensor("beta_r", [P, CCH], F32, kind="ExternalInput")
    pb_d = nc.dram_tensor("pb_r", [P, CCH], F32, kind="ExternalInput")
    ones_d = nc.dram_tensor("ones_b", [P, 2 * P], FP8, kind="ExternalInput")
    g_d = nc.dram_tensor("gmat", [P, 8], F32, kind="ExternalInput")
    g2_d = nc.dram_tensor("g2mat", [8, P], F32, kind="ExternalInput")
    if use_amt:
        amtw_d = nc.dram_tensor("amtw", [P, CCH], FP8, kind="ExternalInput")
    out_d = nc.dram_tensor("out", [C, N], F32, kind="ExternalOutput")

    with tile.TileContext(nc) as tc:
        # ---- persistent pools ----
        const = tc.alloc_tile_pool(name="const", bufs=1)
        xs_pool = tc.alloc_tile_pool(name="xs_pool", bufs=2)
        r8_pool = tc.alloc_tile_pool(name="r8_pool", bufs=1)
        u8_pool = tc.alloc_tile_pool(name="u8_pool", bufs=1)
        vt_pool = tc.alloc_tile_pool(name="vt_pool", bufs=NPAIR)
        et_pool = tc.alloc_tile_pool(name="et_pool", bufs=12)
        xr_pool = tc.alloc_tile_pool(name="xr_pool", bufs=8)
        tt_pool = tc.alloc_tile_pool(name="tt_pool", bufs=4)
        ob_pool = tc.alloc_tile_pool(name="ob_pool", bufs=4)
        rd_pool = tc.alloc_tile_pool(name="rd_pool", bufs=2)
        bn_pool = tc.alloc_tile_pool(name="bn_pool", bufs=2)
        st_pool = tc.alloc_tile_pool(name="st_pool", bufs=2)
        dd_pool = tc.alloc_tile_pool(name="dd_pool", bufs=2)

        wst_sb = const.tile([P, CCH, NW], FP8)
        wvt_sb = const.tile([P, CCH, NW], FP8)
        gamma_sb = const.tile([P, CCH], F32)
        beta_sb = const.tile([P, CCH], F32)
        pb_sb = const.tile([P, CCH], F32)
        ones_sb = const.tile([P, 2, P], FP8)
        onesf_sb = const.tile([P, P], BF16)
        g_sb = const.tile([P, 8], F32)
        g2_sb = const.tile([8, P], F32)
        for cp in range(CCH):
            nc.sync.dma_start(out=wst_sb[:, cp, :], in_=wst_d[cp * P:(cp + 1) * P, :])
            nc.sync.dma_start(out=wvt_sb[:, cp, :], in_=wvt_d[cp * P:(cp + 1) * P, :])
        nc.sync.dma_start(out=gamma_sb, in_=gamma_d[:, :])
        nc.sync.dma_start(out=beta_sb, in_=beta_d[:, :])
        nc.sync.dma_start(out=pb_sb, in_=pb_d[:, :])
        nc.sync.dma_start(out=ones_sb, in_=ones_d[:, :].rearrange('p (a b) -> p a b', a=2))
        nc.vector.memset(onesf_sb, 1.0)
        nc.sync.dma_start(out=g_sb, in_=g_d[:, :])
        nc.sync.dma_start(out=g2_sb, in_=g2_d[:, :])
        if use_amt:
            amtw_sb = const.tile([P, CCH, 1], FP8)
            nc.sync.dma_start(out=amtw_sb[:, :, 0], in_=amtw_d[:, :])
            amt_sb = const.tile([P, MT], F32)

        # ================= per-rep emission =================

        def emit_rep():
            # ---------- 1. GroupNorm head ----------
            r8_sb = r8_pool.tile([P, CCH, N], FP8, tag="r8", name="r8")
            with tc.tile_pool(name="psg", bufs=1, space="PSUM") as psg_pool:
                x1s, s_sbs = [], []
                for cp in range(CCH):
                    x1 = xs_pool.tile([P, N], F32, tag="x1", name="x1")
                    for s in range(8):
                        nc.sync.dma_start(
                            out=x1[:, s * NW:(s + 1) * NW],
                            in_=x_in[cp * P:(cp + 1) * P, s * NW:(s + 1) * NW])
                    x1s.append(x1)
                for cp in range(CCH):
                    bnst = bn_pool.tile([P, 8, 6], F32, tag="bnst")
                    for s in range(8):
                        nc.vector.bn_stats(out=bnst[:, s, :],
                                           in_=x1s[cp][:, s * NW:(s + 1) * NW])
                    mv = bn_pool.tile([P, 2], F32, tag="mv")
                    nc.vector.bn_aggr(out=mv, in_=bnst)
                    # per-partition [mean, E[x^2]]
                    s_sb = bn_pool.tile([P, 2], F32, tag=f"s_sb{cp}")
                    nc.vector.tensor_copy(out=s_sb[:, 0:1], in_=mv[:, 0:1])
                    nc.vector.scalar_tensor_tensor(
                        out=s_sb[:, 1:2], in0=mv[:, 0:1], scalar=mv[:, 0:1],
                        in1=mv[:, 1:2], op0=OP.mult, op1=OP.add)
                    s_sbs.append(s_sb)
                for cp in range(CCH):
                    # group-aggregate via PE, rsqrt chain on DVE, broadcast
                    # back via PE, then normalize x -> r8 fp8
                    psg = psg_pool.tile([8, 2], F32, tag="psg", name="psg")
                    nc.tensor.matmul(psg, lhsT=g_sb, rhs=s_sbs[cp],
                                     start=True, stop=True)
                    mu = st_pool.tile([8, 1], F32, tag="mu")
                    nc.vector.tensor_scalar_mul(out=mu, in0=psg[:, 0:1],
                                                scalar1=1.0 / GSIZE)
                    ex2 = st_pool.tile([8, 1], F32, tag="ex2")
                    nc.vector.tensor_scalar_mul(out=ex2, in0=psg[:, 1:2],
                                                scalar1=1.0 / GSIZE)
                    musq = st_pool.tile([8, 1], F32, tag="musq")
                    nc.vector.tensor_mul(out=musq, in0=mu, in1=mu)
                    veps = st_pool.tile([8, 1], F32, tag="veps")
                    nc.vector.scalar_tensor_tensor(
                        out=veps, in0=ex2, scalar=EPS, in1=musq,
                        op0=OP.add, op1=OP.subtract)
                    # rsqrt seed: y0 = bits(0x5F3759DF - (v >> 1))
                    h_i = st_pool.tile([8, 1], I32, tag="h_i")
                    nc.vector.tensor_scalar(
                        out=h_i, in0=veps[:, :].bitcast(I32), scalar1=1,
                        scalar2=None, op0=OP.arith_shift_right)
                    y0_i = st_pool.tile([8, 1], I32, tag="y0_i")
                    nc.vector.tensor_scalar(
                        out=y0_i, in0=h_i, scalar1=-1, scalar2=0x5F3759DF,
                        op0=OP.mult, op1=OP.add)
                    y = y0_i[:, :].bitcast(F32)
                    for it in range(2):
                        t1 = st_pool.tile([8, 1], F32, tag=f"t1_{it}")
                        nc.vector.tensor_mul(out=t1, in0=y, in1=y)
                        t2 = st_pool.tile([8, 1], F32, tag=f"t2_{it}")
                        nc.vector.tensor_mul(out=t2, in0=t1, in1=veps)
                        t3 = st_pool.tile([8, 1], F32, tag=f"t3_{it}")
                        nc.vector.tensor_scalar(
                            out=t3, in0=t2, scalar1=-0.5, scalar2=1.5,
                            op0=OP.mult, op1=OP.add)
                        yn = st_pool.tile([8, 1], F32, tag=f"yn_{it}")
                        nc.vector.tensor_mul(out=yn, in0=t3, in1=y)
                        y = yn
                    w_sb = st_pool.tile([8, 2], F32, tag="w_sb")
                    nc.vector.tensor_copy(out=w_sb[:, 0:1], in_=y)
                    nc.vector.tensor_copy(out=w_sb[:, 1:2], in_=mu)
                    psp2 = psg_pool.tile([P, 2], F32, tag="psg", name="psp2")
                    nc.tensor.matmul(psp2, lhsT=g2_sb, rhs=w_sb,
                                     start=True, stop=True)
                    a_c = st_pool.tile([P, 1], F32, tag=f"a_c{cp}")
                    nc.vector.tensor_mul(out=a_c, in0=gamma_sb[:, cp:cp + 1],
                                         in1=psp2[:, 0:1])
                    tb = st_pool.tile([P, 1], F32, tag="tb")
                    nc.vector.tensor_mul(out=tb, in0=psp2[:, 1:2], in1=a_c)
                    b_c = st_pool.tile([P, 1], F32, tag=f"b_c{cp}")
                    nc.vector.tensor_sub(out=b_c, in0=beta_sb[:, cp:cp + 1],
                                         in1=tb)
                    ab_cs.append((a_c, b_c))
                # normalize in column quarters, all channels of a quarter
                # first, so the U/V' matmuls (which need all 4 channel chunks
                # of a column window) can start after the first quarter
                # instead of after the full 4096 columns
                NQ = N // 4
                for q in range(4):
                    for cp in range(CCH):
                        a_c, b_c = ab_cs[cp]
                        nc.vector.tensor_scalar(
                            out=r8_sb[:, cp, q * NQ:(q + 1) * NQ],
                            in0=x1s[cp][:, q * NQ:(q + 1) * NQ],
                            scalar1=a_c, scalar2=b_c,
                            op0=OP.mult, op1=OP.add)

            # ---------- 2. U / V' phase (multi-bank staging) ----------
            u8_sb = u8_pool.tile([P, CCH, N], FP8, tag="u8", name="u8")
            vt_sb = []
            with tc.tile_pool(name="psv", bufs=6, space="PSUM") as psv_pool:
                copy_i = [0]

                def evac(dst, src):
                    # alternate evacuation engine so neither paces PE
                    if copy_i[0] % 2 == 0:
                        nc.vector.tensor_copy(out=dst, in_=src)
                    else:
                        nc.scalar.activation(out=dst, in_=src, func=AF.Copy)
                    copy_i[0] += 1

                def emit_u(cq, mc):
                    psv = psv_pool.tile([P, NW], F32, tag="psv", name="psv")
                    for ks in (0, 2):
                        nc.tensor.matmul(
                            psv,
                            lhsT=wst_sb[:, ks:ks + 2, cq * P:(cq + 1) * P],
                            rhs=r8_sb[:, ks:ks + 2, mc * NW:(mc + 1) * NW],
                            start=(ks == 0), stop=(ks == 2), perf_mode=DR)
                    evac(u8_sb[:, cq, mc * NW:(mc + 1) * NW], psv)

                def emit_v(mt):
                    if mt % 2 == 0:
                        vt_sb.append(vt_pool.tile([P, 2, NW], FP8, tag="vt",
                                                  name="vt"))
                    psv = psv_pool.tile([P, NW], F32, tag="psv", name="psv")
                    for ks in (0, 2):
                        nc.tensor.matmul(
                            psv,
                            lhsT=r8_sb[:, ks:ks + 2, mt * P:(mt + 1) * P],
                            rhs=wvt_sb[:, ks:ks + 2, :],
                            start=(ks == 0), stop=(ks == 2), perf_mode=DR)
                    evac(vt_sb[mt // 2][:, mt % 2, :], psv)
                    if use_amt:
                        psa = psv_pool.tile([P, 1], F32, tag="psa", name="psa")
                        for ks in (0, 2):
                            nc.tensor.matmul(
                                psa,
                                lhsT=r8_sb[:, ks:ks + 2, mt * P:(mt + 1) * P],
                                rhs=amtw_sb[:, ks:ks + 2, :],
                                start=(ks == 0), stop=(ks == 2), perf_mode=DR)
                        nc.vector.tensor_copy(out=amt_sb[:, mt:mt + 1], in_=psa)

                # consumption order: scores consume u8 window mc at pair 2*mc;
                # dpv consumes vt[pt] at pair pt+DEPTH.
                for mc in range(NCH):
                    for cq in range(CCH):
                        emit_u(cq, mc)
                    for mt in (4 * mc, 4 * mc + 1, 4 * mc + 2, 4 * mc + 3):
                        emit_v(mt)

            # ---------- 3. attention ----------
            with tc.tile_pool(name="pss", bufs=SCORE_BANKS, space="PSUM") as pss_pool, \
                 tc.tile_pool(name="pso", bufs=1, space="PSUM") as pso_pool, \
                 tc.tile_pool(name="psd", bufs=1, space="PSUM") as psd_pool:

                def emit_dpv(et_t, pt, psd_t, pso_tiles, first, last):
                    if not DVE_D:
                        nc.tensor.matmul(psd_t, lhsT=ones_sb, rhs=et_t,
                                         start=first, stop=last, perf_mode=DR)
                    for cs in range(CCH):
                        nc.tensor.matmul(
                            pso_tiles[cs],
                            lhsT=vt_sb[pt][:, :, cs * P:(cs + 1) * P],
                            rhs=et_t, start=first, stop=last, perf_mode=DR)

                xr_tiles = {}

                def emit_out(state, cs):
                    pso_tiles, rd_t, pnch = state
                    t_t = tt_pool.tile([P, NW], F32, tag="t_t")
                    nc.vector.tensor_mul(out=t_t, in0=pso_tiles[cs], in1=rd_t)
                    ob = ob_pool.tile([P, NW], F32, tag="ob")
                    nc.vector.scalar_tensor_tensor(
                        out=ob, in0=t_t, scalar=pb_sb[:, cs:cs + 1],
                        in1=xr_tiles.pop((pnch, cs)),
                        op0=OP.add, op1=OP.add)
                    nc.sync.dma_start(
                        out=out_d[cs * P:(cs + 1) * P, pnch * NW:(pnch + 1) * NW],
                        in_=ob)

                state = None
                dstate = None

                def close_d(pso_tiles_p, d_acc_p, psd_t_p, pnch):
                    # d-close: one small matmul broadcasts the partition-sum
                    # of the DVE denominator partials to all 128 rows
                    rd_t = rd_pool.tile([P, NW], F32, tag="rd")
                    if DVE_D:
                        psd2 = psd_pool.tile([P, NW], F32, tag="psd")
                        nc.tensor.matmul(psd2, lhsT=onesf_sb, rhs=d_acc_p,
                                         start=True, stop=True)
                        nc.vector.reciprocal(out=rd_t, in_=psd2)
                    else:
                        nc.vector.reciprocal(out=rd_t, in_=psd_t_p)
                    return (pso_tiles_p, rd_t, pnch)

                out_slots = {1: 0, 2: 1, 3: 2, 4: 3}
                xr_slots = {16: 0, 18: 1, 20: 2, 22: 3}
                for nch in range(NCH):
                    pso_tiles = [pso_pool.tile([P, NW], F32, tag=f"pso{cs}",
                                               name=f"pso{cs}")
                                 for cs in range(CCH)]
                    psd_t = None
                    if not DVE_D:
                        psd_t = psd_pool.tile([P, NW], F32, tag="psd")
                    d_acc = None
                    pend = []
                    cur_et = None
                    pss_pair = [None, None]
                    for mt in range(MT):
                        half = mt % 2
                        if half == 0:
                            # pair of score banks; ks-passes interleaved so
                            # consecutive matmuls never hit the same PSUM
                            # bank back-to-back (HW RMW hazard)
                            pss_pair[0] = pss_pool.tile([P, NW], F32,
                                                        tag="pss", name="pssA")
                            pss_pair[1] = pss_pool.tile([P, NW], F32,
                                                        tag="pss", name="pssB")
                            for ks in (0, 2):
                                for h in (0, 1):
                                    nc.tensor.matmul(
                                        pss_pair[h],
                                        lhsT=u8_sb[:, ks:ks + 2,
                                                   (mt + h) * P:(mt + h + 1) * P],
                                        rhs=r8_sb[:, ks:ks + 2,
                                                  nch * NW:(nch + 1) * NW],
                                        start=(ks == 0), stop=(ks == 2),
                                        perf_mode=DR)
                            cur_et = et_pool.tile([P, 2, NW], FP8, tag="et",
                                                  name="et")
                        pss = pss_pair[half]
                        if use_amt:
                            nc.scalar.activation(out=cur_et[:, half, :], in_=pss,
                                                 func=AF.Exp, scale=INV_SQRT_C,
                                                 bias=amt_sb[:, mt:mt + 1])
                        else:
                            nc.scalar.activation(out=cur_et[:, half, :], in_=pss,
                                                 func=AF.Exp, scale=INV_SQRT_C)
                        if DVE_D and half == 1:
                            # denominator partials on DVE, off the ring.
                            # bf16 accumulation keeps the adds in the DVE 2x
                            # perf mode; d's rounding error (~0.1-0.3% rel)
                            # scales whole output columns and sits well
                            # inside the error budget.
                            if d_acc is None:
                                d_acc = dd_pool.tile([P, NW], BF16, tag="d_acc0")
                                nc.vector.scalar_tensor_tensor(
                                    out=d_acc, in0=cur_et[:, 0, :], scalar=1.0,
                                    in1=cur_et[:, 1, :], op0=OP.mult, op1=OP.add)
                            else:
                                psum_t = dd_pool.tile([P, NW], BF16, tag="ps_d")
                                nc.vector.scalar_tensor_tensor(
                                    out=psum_t, in0=cur_et[:, 0, :], scalar=1.0,
                                    in1=cur_et[:, 1, :], op0=OP.mult, op1=OP.add)
                                d_new = dd_pool.tile([P, NW], BF16,
                                                     tag=f"d_acc{(mt // 2) % 2}")
                                nc.vector.tensor_add(out=d_new, in0=d_acc,
                                                     in1=psum_t)
                                d_acc = d_new
                        if mt == 0 and dstate is not None:
                            # deferred d-close of the previous n-chunk: by now
                            # the DVE d-chain tail has drained behind pair 0's
                            # score matmuls, so PE doesn't stall on it
                            state = close_d(*dstate)
                            dstate = None
                        if state is not None and mt in out_slots:
                            emit_out(state, out_slots[mt])
                        if mt in xr_slots:
                            cs = xr_slots[mt]
                            xr = xr_pool.tile([P, NW], F32, tag="xr")
                            nc.sync.dma_start(
                                out=xr,
                                in_=x_in[cs * P:(cs + 1) * P,
                                         nch * NW:(nch + 1) * NW])
                            xr_tiles[(nch, cs)] = xr
                        if half == 1:
                            pend.append((cur_et, mt // 2))
                            if len(pend) > DEPTH:
                                p_et, pt = pend.pop(0)
                                emit_dpv(p_et, pt, psd_t, pso_tiles,
                                         first=(pt == 0), last=False)
                    for p_et, pt in pend:
                        emit_dpv(p_et, pt, psd_t, pso_tiles,
                                 first=(pt == 0), last=(pt == NPAIR - 1))
                    dstate = (pso_tiles, d_acc, psd_t, nch)
                state = close_d(*dstate)
                for cs in range(CCH):
                    emit_out(state, cs)

        for _rep in range(reps):
            emit_rep()

        for pool in (dd_pool, st_pool, bn_pool, rd_pool, ob_pool, tt_pool,
                     xr_pool, et_pool, vt_pool, u8_pool, r8_pool, xs_pool,
                     const):
            pool.release()

    nc.compile()
    return nc


def _prep_inputs(x, gn_gamma, gn_beta, q_w, q_b, k_w, k_b, proj_w, proj_b):
    use_amt = bool(np.any(q_b != 0))

    f8 = ml_dtypes.float8_e4m3
    f64 = np.float64
    ws_t = np.ascontiguousarray((k_w.T.astype(f64) @ q_w.astype(f64))
                                .astype(np.float32).astype(f8))
    # V' = (Wp Wk) R ; wvt = (Wp Wk)^T = Wk^T Wp^T
    wv_t = np.ascontiguousarray((k_w.T.astype(f64) @ proj_w.T.astype(f64))
                                .astype(np.float32).astype(f8))
    gamma_r = np.ascontiguousarray(gn_gamma.reshape(CCH, P).T.astype(np.float32))
    beta_r = np.ascontiguousarray(gn_beta.reshape(CCH, P).T.astype(np.float32))
    # attn rows sum to 1, so Wp kb is a constant channel bias: fold into pb
    pb_eff = (proj_b.astype(f64) + proj_w.astype(f64) @ k_b.astype(f64)).astype(
        np.float32)
    pb_r = np.ascontiguousarray(pb_eff.reshape(CCH, P).T)
    ones_b = np.ones((P, 2 * P), dtype=f8)
    gmat = np.zeros((P, 8), dtype=np.float32)
    gmat[np.arange(P), np.arange(P) // GSIZE] = 1.0
    g2mat = np.ascontiguousarray(gmat.T)

    common = {
        "wst": ws_t, "wvt": wv_t,
        "gamma_r": gamma_r, "beta_r": beta_r, "pb_r": pb_r,
        "ones_b": ones_b, "gmat": gmat, "g2mat": g2mat,
    }
    if use_amt:
        # amt[m] = qb . (Wk r_m) = (Wk^T qb) . r_m, pre-scaled by 1/sqrt(C);
        # the qb.kb term is constant over m AND n -> cancels in softmax.
        w_vec = (k_w.T.astype(f64) @ q_b.astype(f64)).astype(np.float32)
        w_vec = w_vec * INV_SQRT_C
        common["amtw"] = np.ascontiguousarray(
            w_vec.reshape(CCH, P).T.astype(f8))

    in_maps = []
    for i in range(B):
        m = dict(common)
        m["x_in"] = np.ascontiguousarray(x[i].reshape(C, N).astype(np.float32))
        in_maps.append(m)
    return in_maps, use_amt, False


def kernel(x, gn_gamma, gn_beta, q_w, q_b, k_w, k_b, proj_w, proj_b, _trace=False):
    x = np.asarray(x)
    in_maps, use_amt, use_kb = _prep_inputs(
        x, np.asarray(gn_gamma), np.asarray(gn_beta), np.asarray(q_w),
        np.asarray(q_b), np.asarray(k_w), np.asarray(k_b),
        np.asarray(proj_w), np.asarray(proj_b))

    key = (use_amt, use_kb)
    if key not in _BUILD_CACHE:
        _BUILD_CACHE[key] = _build(use_amt, use_kb)
    nc = _BUILD_CACHE[key]

    res = bass_utils.run_bass_kernel_spmd(
        nc, in_maps, core_ids=list(range(B)), trace=_trace)
    out = np.stack([r["out"].reshape(C, 64, 64) for r in res.results])
    kernel.last_result = res
    return out.astype(x.dtype)


def make_runner(inputs, chain=1):
    """Build the jitted 8-core executable once; return a callable that runs it
    once and returns wall ns, plus a decoder for the outputs."""
    import time
    import jax
    from jax.experimental.shard_map import shard_map
    from jax.sharding import Mesh, PartitionSpec
    from concourse import bass2jax
    import concourse.mybir as mb

    in_maps, use_amt, use_kb = _prep_inputs(
        np.asarray(inputs["x"]), np.asarray(inputs["gn_gamma"]),
        np.asarray(inputs["gn_beta"]), np.asarray(inputs["q_w"]),
        np.asarray(inputs["q_b"]), np.asarray(inputs["k_w"]),
        np.asarray(inputs["k_b"]), np.asarray(inputs["proj_w"]),
        np.asarray(inputs["proj_b"]))
    key = (use_amt, use_kb, chain)
    if key not in _BUILD_CACHE:
        _BUILD_CACHE[key] = _build(use_amt, use_kb, reps=chain)
    nc = _BUILD_CACHE[key]

    bass2jax.install_neuronx_cc_hook()
    partition_name = nc.partition_id_tensor.name if nc.partition_id_tensor else None
    in_names, out_names, out_avals, zero_outs = [], [], [], []
    for alloc in nc.m.functions[0].allocations:
        if not isinstance(alloc, mb.MemoryLocationSet):
            continue
        name = alloc.memorylocations[0].name
        if alloc.kind == "ExternalInput":
            if name != partition_name:
                in_names.append(name)
        elif alloc.kind == "ExternalOutput":
            out_names.append(name)
            shape = tuple(alloc.tensor_shape)
            dtype = mb.dt.np(alloc.dtype)
            out_avals.append(jax.core.ShapedArray(shape, dtype))
            zero_outs.append(np.zeros(shape, dtype))
    n_params = len(in_names)
    n_outs = len(out_avals)
    all_names = in_names + out_names
    if partition_name is not None:
        all_names = all_names + [partition_name]

    def _body(*args):
        operands = list(args)
        if partition_name is not None:
            operands.append(bass2jax.partition_id_tensor())
        outs = bass2jax._bass_exec_p.bind(
            *operands,
            out_avals=tuple(out_avals),
            in_names=tuple(all_names),
            out_names=tuple(out_names),
            lowering_input_output_aliases=(),
            sim_require_finite=True,
            sim_require_nnan=True,
            nc=nc,
        )
        return tuple(outs)

    donate = tuple(range(n_params, n_params + n_outs))
    devices = jax.devices()[:B]
    mesh = Mesh(np.asarray(devices), ("core",))
    sharded = jax.jit(
        shard_map(_body, mesh=mesh,
                  in_specs=(PartitionSpec("core"),) * (n_params + n_outs),
                  out_specs=(PartitionSpec("core"),) * n_outs,
                  check_rep=False),
        donate_argnums=donate, keep_unused=True)

    concat_in = [
        np.concatenate([np.asarray(in_maps[c][nm]) for c in range(B)], axis=0)
        for nm in in_names
    ]
    concat_zeros = [
        np.zeros((B * z.shape[0], *z.shape[1:]), z.dtype) for z in zero_outs
    ]
    sharding = jax.sharding.NamedSharding(mesh, PartitionSpec("core"))
    dev_in = [jax.device_put(a, sharding) for a in concat_in]

    state = {}

    def run_once():
        # the kernel fully overwrites its outputs, so recycle the previous
        # call's output arrays as the next call's donated buffers -- no
        # per-call H2D transfer (which dominated dispatch jitter)
        outs = state.get("out_arrs")
        if outs is None:
            outs = [jax.device_put(z, sharding) for z in concat_zeros]
        for z in outs:
            z.block_until_ready()
        t0 = time.perf_counter()
        out_arrs = sharded(*dev_in, *outs)
        for o in out_arrs:
            o.block_until_ready()
        dt = (time.perf_counter() - t0) * 1e9
        state["out_arrs"] = out_arrs
        return dt

    def decode():
        out_arrs = state["out_arrs"]
        return [
            {nm: np.asarray(out_arrs[i]).reshape(B, *out_avals[i].shape)[c]
             for i, nm in enumerate(out_names)}
            for c in range(B)
        ]

    return run_once, decode


def bench(inputs, iters=6, chain=1):
    run_once, decode = make_runner(inputs, chain=chain)
    times = [run_once() for _ in range(iters)]
    return min(times), times, decode()
